# revision 16
# baseline (speedup 1.0000x reference)
"""Trainium2 Bass kernel for nn_MetaNetLinearizedModel (v6: no-collective
F-sharding + fp8 DoubleRow U-stream).

Each core owns a 96-column slice fc of the feature dim F=768 and computes,
fully locally (no AllReduce):
    z1_c  = X @ W1[:, fc]                      (bf16, f32 accum)
    g_c   = gelu(z1_c + b1[fc]) ;  gp_c = gelu'(...) via central difference
    U_t,c = X @ dW1[t][:, fc]                  (fp8 x fp8 DoubleRow)
    v_t,c = gp_c * (U_t,c + db1[t][fc])
    P_t,c = v_t,c @ W2[fc, :] + g_c @ dW2[t][fc, :]    -> PO rows 16t:16t+16
    fo_c  = g_c @ W2[fc, :]                            -> FO (feats partial)
The host sums partials across cores, runs the tiny meta-net for coefs, and
forms  out = feats + b2 + sum_t coefs[:,t] * P_t + coefs @ db2.

DoubleRow fp8 matmuls consume two k-tiles per instruction (2 cols/cycle),
halving both PE array time and instruction count for the dominant U
stream.  DMA transfers are few and large at the head of each ring (the
queue engines stall when many small transfers rotate through the limited
DMA-completion semaphores), with quarter-size final dW1 chunks so the
end-of-stream matmul backlog stays short.  Each half's reduce/v-term tail
and its half of the PO export drain while the other ring still streams.
"""
import sys

sys.path.insert(0, "/opt/trn_rl_repo")

import numpy as np
import ml_dtypes
import concourse.bass as bass
import concourse.bacc as bacc
import concourse.tile as tile
import concourse.mybir as mybir
from concourse import bass_utils

F32 = mybir.dt.float32
BF16 = mybir.dt.bfloat16
FP8 = mybir.dt.float8e4
AF = mybir.ActivationFunctionType
OP = mybir.AluOpType
PM = mybir.MatmulPerfMode

B = 16
D = 3 * 64 * 64        # 12288
F = 768
HID = 192
T = 8
NCORES = 8
FSH = F // NCORES      # 96 columns of F per core
KD = D // 128          # 96 k-tiles
FP8_SCALE = 32.0       # dW1 fp8 scale
FP8_XS = 4.0           # X fp8 scale (U stream stationary)
EPS = 0.125            # central-difference step for gelu'
# v = gp*(U+db1):  U_psum = (4 X)@(32 dW1) = 128 U, db1r = 128 db1,
# gp = 4 (Gp - Gm)  =>  fold 4/128 into gpd
GPD_S = 4.0 / (FP8_SCALE * FP8_XS)

HW = 4 * FSH           # 384 dW1 columns per k-tile per half (4 tasks)
W1KA = 72              # w1 slice k-tiles in the sync-ring transfer
# dW1 chunk sizes (k-tiles, all even for DoubleRow pairs) per half
DWCH = [12] * 7 + [8, 2, 2]
DWOF = [sum(DWCH[:j]) for j in range(len(DWCH) + 1)]
NDWC = len(DWCH)

_CACHE = {}


def build():
    nc = bacc.Bacc("TRN2", target_bir_lowering=False, debug=False,
                   enable_asserts=False, num_devices=NCORES)

    XT = nc.dram_tensor("xt", [128, KD * B], BF16, kind="ExternalInput")
    XTF8 = nc.dram_tensor("xtf8", [128, KD * B], FP8, kind="ExternalInput")
    W1S = nc.dram_tensor("w1s", [128, KD * FSH], BF16, kind="ExternalInput")
    DW1A = nc.dram_tensor("dw1a", [128, KD * HW], FP8, kind="ExternalInput")
    DW1B = nc.dram_tensor("dw1b", [128, KD * HW], FP8, kind="ExternalInput")
    WD2 = nc.dram_tensor("wd2", [FSH, (T + 1) * F], BF16,
                         kind="ExternalInput")
    CONS = nc.dram_tensor("cons", [128, 35], F32, kind="ExternalInput")
    DB1R = nc.dram_tensor("db1r", [B, F], F32, kind="ExternalInput")
    PO = nc.dram_tensor("po", [128, F], BF16, kind="ExternalOutput")
    FO = nc.dram_tensor("fo", [B, F], F32, kind="ExternalOutput")

    with tile.TileContext(nc, num_cores=NCORES) as tc:
        with (
            tc.tile_pool(name="cst", bufs=1) as cst,
            tc.tile_pool(name="dwa", bufs=1) as dwap,
            tc.tile_pool(name="dwb", bufs=1) as dwbp,
            tc.tile_pool(name="wrk", bufs=1) as wrk,
            tc.tile_pool(name="psa", bufs=1, space="PSUM") as psa,
            tc.tile_pool(name="psb", bufs=1, space="PSUM") as psb,
            tc.tile_pool(name="psu", bufs=1, space="PSUM") as psu,
            tc.tile_pool(name="pss", bufs=2, space="PSUM") as pss,
        ):
            # ---- activation LUT preload (gelu table resident early) ----
            scr = wrk.tile([1, 2], F32)
            nc.vector.memset(scr[:], 0.0)
            scr2 = wrk.tile([1, 2], F32)
            nc.scalar.activation(scr2[:, 0:1], scr[:, 0:1], AF.Gelu_apprx_tanh)

            # ---- DMA kicks: few, large transfers per ring ----
            # sync ring:   xt, w1s[0:72], dW1A chunks        (~6.9 MB)
            # scalar ring: cons, db1r, w1s[72:96], wd2, dW1B (~6.7 MB)
            # gpsimd:      xtf8 in, fo/po out
            xt_sb = cst.tile([128, KD * B], BF16)
            nc.sync.dma_start(xt_sb[:], XT.ap())
            w1a_sb = cst.tile([128, W1KA * FSH], BF16)
            nc.sync.dma_start(w1a_sb[:], W1S.ap()[:, 0:W1KA * FSH])
            dwa = []
            for j in range(NDWC):
                t_ = dwap.tile([128, DWCH[j] * HW], FP8, name="dwat",
                               tag=f"dwat{j}")
                nc.sync.dma_start(
                    t_[:], DW1A.ap()[:, DWOF[j] * HW:DWOF[j + 1] * HW])
                dwa.append(t_)

            xtf8_sb = cst.tile([128, KD * B], FP8)
            nc.gpsimd.dma_start(xtf8_sb[:], XTF8.ap())
            cons_sb = cst.tile([128, 35], F32)
            nc.scalar.dma_start(cons_sb[:], CONS.ap())
            db1r_sb = cst.tile([B, F], F32)
            nc.scalar.dma_start(db1r_sb[:], DB1R.ap())
            w1b_sb = cst.tile([128, (KD - W1KA) * FSH], BF16)
            nc.scalar.dma_start(w1b_sb[:], W1S.ap()[:, W1KA * FSH:KD * FSH])
            wd2_sb = cst.tile([FSH, (T + 1) * F], BF16)
            nc.scalar.dma_start(wd2_sb[:], WD2.ap())
            dwb = []
            for j in range(NDWC):
                t_ = dwbp.tile([128, DWCH[j] * HW], FP8, name="dwbt",
                               tag=f"dwbt{j}")
                nc.scalar.dma_start(
                    t_[:], DW1B.ap()[:, DWOF[j] * HW:DWOF[j + 1] * HW])
                dwb.append(t_)

            eye = cons_sb[0:32, 0:32]
            b1c = cons_sb[0:FSH, 32:33]
            b1p = cons_sb[0:FSH, 33:34]
            b1m = cons_sb[0:FSH, 34:35]
            w2_sb = wd2_sb[:, 0:F]
            xtf8_3 = xtf8_sb[:].rearrange("p (k b) -> p k b", b=B)

            # ---- z1 = X @ W1[:, fc]  (single accumulation group) ----
            z1ps = pss.tile([B, FSH], F32, name="sp", tag="sp",
                            padded_shape=[128, 512])
            for k in range(KD):
                if k < W1KA:
                    wv = w1a_sb[:, k * FSH:(k + 1) * FSH]
                else:
                    wv = w1b_sb[:, (k - W1KA) * FSH:(k - W1KA + 1) * FSH]
                nc.tensor.matmul(z1ps[:], xt_sb[:, k * B:(k + 1) * B], wv,
                                 start=(k == 0), stop=(k == KD - 1),
                                 skip_group_check=True)

            # z1 -> f-major [f, b] via PE transpose
            z1pad = wrk.tile([32, FSH], F32)
            nc.vector.tensor_copy(z1pad[0:B, :], z1ps[:])
            z1tp = pss.tile([FSH, 32], F32, name="sp", tag="sp",
                            padded_shape=[128, 512])
            nc.tensor.matmul(z1tp[:], z1pad[:], eye, is_transpose=True,
                             skip_group_check=True)
            z1t = wrk.tile([FSH, B], F32)
            nc.vector.tensor_copy(z1t[:], z1tp[:, 0:B])

            # g = gelu(z1+b1); gpd = (G(z1+b1+eps) - G(z1+b1-eps)) * GPD_S
            gT = wrk.tile([FSH, B], BF16)
            nc.scalar.activation(gT[:], z1t[:], AF.Gelu_apprx_tanh, bias=b1c)
            gpp = wrk.tile([FSH, B], F32)
            nc.scalar.activation(gpp[:], z1t[:], AF.Gelu_apprx_tanh, bias=b1p)
            gpm = wrk.tile([FSH, B], F32)
            nc.scalar.activation(gpm[:], z1t[:], AF.Gelu_apprx_tanh, bias=b1m)
            gpd = wrk.tile([FSH, B], F32)
            nc.vector.tensor_sub(gpd[:], gpp[:], gpm[:])
            nc.vector.tensor_scalar_mul(gpd[:], gpd[:], GPD_S)

            # pair stationaries for the g @ dW2 term: [g|0] and [0|g]
            gpe = wrk.tile([FSH, 32], BF16)
            nc.vector.memset(gpe[:], 0.0)
            gpo = wrk.tile([FSH, 32], BF16)
            nc.vector.memset(gpo[:], 0.0)
            nc.vector.tensor_copy(gpe[:, 0:16], gT[:])
            nc.vector.tensor_copy(gpo[:, 16:32], gT[:])

            # ---- feats partial: fo = g @ W2[fc, :] ----
            fps5 = pss.tile([B, 512], F32, name="sp", tag="sp",
                            padded_shape=[128, 512])
            nc.tensor.matmul(fps5[:], gT[:], w2_sb[:, 0:512],
                             start=True, stop=True, skip_group_check=True)
            fps2 = pss.tile([B, 256], F32, name="sp", tag="sp",
                            padded_shape=[128, 512])
            nc.tensor.matmul(fps2[:], gT[:], w2_sb[:, 512:F],
                             start=True, stop=True, skip_group_check=True)
            fo_sb = wrk.tile([B, F], F32)
            nc.vector.tensor_copy(fo_sb[:, 0:512], fps5[:])
            nc.vector.tensor_copy(fo_sb[:, 512:F], fps2[:])
            nc.gpsimd.dma_start(FO.ap(), fo_sb[:])

            # ---- P accumulation: pair group j owns rows 32j:32j+32 ----
            P5 = psu.tile([128, 512], F32, name="p5")
            P2 = psu.tile([128, 256], F32, name="p2",
                          padded_shape=[128, 512])
            vps = wrk.tile([FSH, 128], BF16)
            po_sb = wrk.tile([128, F], BF16)

            def mm_gterm(t):
                j = t // 2
                st = gpe if t % 2 == 0 else gpo
                mv = wd2_sb[:, (1 + t) * F:(1 + t) * F + 512]
                nc.tensor.matmul(P5[32 * j:32 * j + 32, :], st[:], mv,
                                 start=(t % 2 == 0), stop=False,
                                 tile_position=(0, 32 * j),
                                 skip_group_check=True)
                mv = wd2_sb[:, (1 + t) * F + 512:(2 + t) * F]
                nc.tensor.matmul(P2[32 * j:32 * j + 32, :], st[:], mv,
                                 start=(t % 2 == 0), stop=False,
                                 tile_position=(0, 32 * j),
                                 skip_group_check=True)

            for t in range(T):
                mm_gterm(t)

            # ---- U streams (DoubleRow fp8): two k-pair chains per half,
            # each in its own PSUM bank at partition 0 (the ISA requires
            # dst partition 0 for DoubleRow)
            upsA = [psa.tile([B, HW], F32, name=f"ua{i}",
                             padded_shape=[128, 512]) for i in range(2)]
            upsB = [psb.tile([B, HW], F32, name=f"ub{i}",
                             padded_shape=[128, 512]) for i in range(2)]

            def chunk_u(ups, chunks, j):
                ch3 = chunks[j][:].rearrange("p (k f) -> p k f", f=HW)
                for lp in range(DWCH[j] // 2):
                    p = DWOF[j] // 2 + lp
                    nc.tensor.matmul(ups[p % 2][:],
                                     xtf8_3[:, 2 * p:2 * p + 2, :],
                                     ch3[:, 2 * lp:2 * lp + 2, :],
                                     start=(p < 2), stop=(p >= KD // 2 - 2),
                                     perf_mode=PM.DoubleRow,
                                     skip_group_check=True)

            def chunkA(j):
                chunk_u(upsA, dwa, j)

            def chunkB(j):
                chunk_u(upsB, dwb, j)

            def drain_half(ups, tbase):
                # (U0 + 128*db1) + U1, transpose, * gelu'-scale; each add
                # reads one PSUM operand (hardware limit) and one SBUF one
                for tt in range(4):
                    t = tbase + tt
                    tsum = wrk.tile([B, FSH], F32, name="ts", tag="ts",
                                    bufs=2)
                    nc.vector.tensor_add(
                        tsum[:],
                        ups[0][:, tt * FSH:(tt + 1) * FSH],
                        db1r_sb[:, t * FSH:(t + 1) * FSH])
                    tzpad = wrk.tile([32, FSH], F32, name="tz", tag="tz",
                                     bufs=2)
                    nc.vector.tensor_add(
                        tzpad[0:B, :],
                        ups[1][:, tt * FSH:(tt + 1) * FSH],
                        tsum[:])
                    tztp = pss.tile([FSH, 32], F32, name="sp", tag="sp",
                                    padded_shape=[128, 512])
                    nc.tensor.matmul(tztp[:], tzpad[:], eye,
                                     is_transpose=True,
                                     skip_group_check=True)
                    nc.vector.tensor_mul(vps[:, t * B:(t + 1) * B],
                                         tztp[:, 0:B], gpd[:])
                ro = tbase * B
                nc.tensor.matmul(P5[ro:ro + 64, :], vps[:, ro:ro + 64],
                                 w2_sb[:, 0:512],
                                 start=False, stop=True,
                                 tile_position=(0, ro),
                                 skip_group_check=True)
                nc.tensor.matmul(P2[ro:ro + 64, :], vps[:, ro:ro + 64],
                                 w2_sb[:, 512:F],
                                 start=False, stop=True,
                                 tile_position=(0, ro),
                                 skip_group_check=True)
                # export this half's PO rows on its own (now idle) ring
                nc.vector.tensor_copy(po_sb[ro:ro + 64, 0:512],
                                      P5[ro:ro + 64, :])
                nc.vector.tensor_copy(po_sb[ro:ro + 64, 512:F],
                                      P2[ro:ro + 64, :])
                eng = nc.scalar if tbase == 4 else nc.sync
                eng.dma_start(PO.ap()[ro:ro + 64, :], po_sb[ro:ro + 64, :])

            # PE emission order tracks expected data-arrival order
            for j in range(8):
                chunkB(j)
                chunkA(j)
            chunkB(8)
            chunkB(9)
            drain_half(upsB, 4)
            chunkA(8)
            chunkA(9)
            drain_half(upsA, 0)

    nc.compile()
    return nc


def _get_nc():
    if "nc" not in _CACHE:
        _CACHE["nc"] = build()
    return _CACHE["nc"]


def _prep_in_maps(x, W1, b1, W2, b2, mW1, mb1, mW2, mb2, dW1, db1, dW2, db2):
    f32 = np.float32
    bf16 = ml_dtypes.bfloat16
    fp8 = ml_dtypes.float8_e4m3
    X = np.ascontiguousarray(np.asarray(x, f32).reshape(B, D))
    XT = np.ascontiguousarray(X.T)
    XTl = XT.reshape(KD, 128, B).transpose(1, 0, 2).reshape(128, KD * B)
    XTb = np.ascontiguousarray(XTl).astype(bf16)
    XTf8 = np.ascontiguousarray(XTl * FP8_XS).astype(fp8)
    W1 = np.asarray(W1, f32)
    W2 = np.asarray(W2, f32)
    b1 = np.asarray(b1, f32)
    dW1 = np.asarray(dW1, f32)
    db1 = np.asarray(db1, f32)
    dW2 = np.asarray(dW2, f32)

    in_maps = []
    for c in range(NCORES):
        fc = slice(c * FSH, (c + 1) * FSH)
        w1s = np.ascontiguousarray(
            W1[:, fc].reshape(KD, 128, FSH).transpose(1, 0, 2)
            .reshape(128, KD * FSH)).astype(bf16)
        dwa = np.ascontiguousarray(
            (dW1[0:4, :, fc] * FP8_SCALE).transpose(1, 0, 2)
            .reshape(KD, 128, HW).transpose(1, 0, 2)
            .reshape(128, KD * HW)).astype(fp8)
        dwb = np.ascontiguousarray(
            (dW1[4:8, :, fc] * FP8_SCALE).transpose(1, 0, 2)
            .reshape(KD, 128, HW).transpose(1, 0, 2)
            .reshape(128, KD * HW)).astype(fp8)
        wd2 = np.concatenate(
            [W2[fc, :][:, None, :],
             dW2[:, fc, :].transpose(1, 0, 2)], axis=1).reshape(FSH,
                                                               (T + 1) * F)
        cons = np.zeros((128, 35), f32)
        cons[0:32, 0:32] = np.eye(32, dtype=f32)
        cons[0:FSH, 32] = b1[fc]
        cons[0:FSH, 33] = b1[fc] + EPS
        cons[0:FSH, 34] = b1[fc] - EPS
        db1r = np.ascontiguousarray(np.broadcast_to(
            (FP8_SCALE * FP8_XS * db1[:, fc]).reshape(T * FSH), (B, F)))
        in_maps.append({
            "xt": XTb,
            "xtf8": XTf8,
            "w1s": w1s,
            "dw1a": dwa,
            "dw1b": dwb,
            "wd2": np.ascontiguousarray(wd2).astype(bf16),
            "cons": cons,
            "db1r": db1r.astype(f32),
        })
    return in_maps


def run(inputs, trace=False, trace_cores=None, tmpdir=None):
    nc = _get_nc()
    in_maps = _prep_in_maps(**inputs)
    res = bass_utils.run_bass_kernel_spmd(
        nc, in_maps, core_ids=list(range(NCORES)), trace=trace,
        trace_cores=trace_cores, tmpdir=tmpdir)

    f64 = np.float64
    b2 = np.asarray(inputs["b2"], f64)
    mW1 = np.asarray(inputs["mW1"], f64)
    mb1 = np.asarray(inputs["mb1"], f64)
    mW2 = np.asarray(inputs["mW2"], f64)
    mb2 = np.asarray(inputs["mb2"], f64)
    db2 = np.asarray(inputs["db2"], f64)

    feats = np.zeros((B, F), f64)
    P = np.zeros((128, F), f64)
    for c in range(NCORES):
        feats += res.results[c]["fo"].astype(f64)
        P += res.results[c]["po"].astype(f64)
    feats += b2[None, :]
    h = np.maximum(feats @ mW1.T + mb1, 0.0)
    coefs = h @ mW2.T + mb2                     # [B, T]
    out = feats + coefs @ db2
    for t in range(T):
        out += coefs[:, t:t + 1] * P[t * B:(t + 1) * B]
    return out.astype(np.float32), res


def kernel(**inputs):
    out, _ = run(inputs, trace=False)
    return out


# revision 17
# speedup vs baseline: 1.0990x; 1.0990x over previous
"""Trainium2 Bass kernel for nn_MetaNetLinearizedModel (v6: no-collective
F-sharding + fp8 DoubleRow U-stream).

Each core owns a 96-column slice fc of the feature dim F=768 and computes,
fully locally (no AllReduce):
    z1_c  = X @ W1[:, fc]                      (bf16, f32 accum)
    g_c   = gelu(z1_c + b1[fc]) ;  gp_c = gelu'(...) via central difference
    U_t,c = X @ dW1[t][:, fc]                  (fp8 x fp8 DoubleRow)
    v_t,c = gp_c * (U_t,c + db1[t][fc])
    P_t,c = v_t,c @ W2[fc, :] + g_c @ dW2[t][fc, :]    -> PO rows 16t:16t+16
    fo_c  = g_c @ W2[fc, :]                            -> FO (feats partial)
The host sums partials across cores, runs the tiny meta-net for coefs, and
forms  out = feats + b2 + sum_t coefs[:,t] * P_t + coefs @ db2.

DoubleRow fp8 matmuls consume two k-tiles per instruction (2 cols/cycle),
halving both PE array time and instruction count for the dominant U
stream.  DMA transfers are few and large at the head of each ring (the
queue engines stall when many small transfers rotate through the limited
DMA-completion semaphores), with quarter-size final dW1 chunks so the
end-of-stream matmul backlog stays short.  Each half's reduce/v-term tail
and its half of the PO export drain while the other ring still streams.
"""
import sys

sys.path.insert(0, "/opt/trn_rl_repo")

import numpy as np
import ml_dtypes
import concourse.bass as bass
import concourse.bacc as bacc
import concourse.tile as tile
import concourse.mybir as mybir
from concourse import bass_utils

F32 = mybir.dt.float32
BF16 = mybir.dt.bfloat16
FP8 = mybir.dt.float8e4
AF = mybir.ActivationFunctionType
OP = mybir.AluOpType
PM = mybir.MatmulPerfMode

B = 16
D = 3 * 64 * 64        # 12288
F = 768
HID = 192
T = 8
NCORES = 8
FSH = F // NCORES      # 96 columns of F per core
KD = D // 128          # 96 k-tiles
FP8_SCALE = 32.0       # dW1 fp8 scale
FP8_XS = 4.0           # X fp8 scale (U stream stationary)
EPS = 0.125            # central-difference step for gelu'
# v = gp*(U+db1):  U_psum = (4 X)@(32 dW1) = 128 U, db1r = 128 db1,
# gp = 4 (Gp - Gm)  =>  fold 4/128 into gpd
GPD_S = 4.0 / (FP8_SCALE * FP8_XS)

HW = 4 * FSH           # 384 dW1 columns per k-tile per half (4 tasks)
W1KA = 72              # w1 slice k-tiles in the sync-ring transfer
# dW1 chunk sizes (k-tiles, all even for DoubleRow pairs) per half
DWCH = [12] * 7 + [8, 2, 2]
DWOF = [sum(DWCH[:j]) for j in range(len(DWCH) + 1)]
NDWC = len(DWCH)

_CACHE = {}


def build():
    nc = bacc.Bacc("TRN2", target_bir_lowering=False, debug=False,
                   enable_asserts=False, num_devices=NCORES)

    XT = nc.dram_tensor("xt", [128, KD * B], BF16, kind="ExternalInput")
    XTF8 = nc.dram_tensor("xtf8", [128, KD * B], FP8, kind="ExternalInput")
    W1S = nc.dram_tensor("w1s", [128, KD * FSH], BF16, kind="ExternalInput")
    DW1A = nc.dram_tensor("dw1a", [128, KD * HW], FP8, kind="ExternalInput")
    DW1B = nc.dram_tensor("dw1b", [128, KD * HW], FP8, kind="ExternalInput")
    WD2 = nc.dram_tensor("wd2", [FSH, (T + 1) * F], BF16,
                         kind="ExternalInput")
    CONS = nc.dram_tensor("cons", [128, 35], F32, kind="ExternalInput")
    DB1R = nc.dram_tensor("db1r", [B, F], F32, kind="ExternalInput")
    PO = nc.dram_tensor("po", [128, F], BF16, kind="ExternalOutput")
    FO = nc.dram_tensor("fo", [B, F], F32, kind="ExternalOutput")

    with tile.TileContext(nc, num_cores=NCORES) as tc:
        with (
            tc.tile_pool(name="cst", bufs=1) as cst,
            tc.tile_pool(name="dwa", bufs=1) as dwap,
            tc.tile_pool(name="dwb", bufs=1) as dwbp,
            tc.tile_pool(name="wrk", bufs=1) as wrk,
            tc.tile_pool(name="psa", bufs=1, space="PSUM") as psa,
            tc.tile_pool(name="psb", bufs=1, space="PSUM") as psb,
            tc.tile_pool(name="psu", bufs=1, space="PSUM") as psu,
            tc.tile_pool(name="pss", bufs=2, space="PSUM") as pss,
        ):
            # ---- activation LUT preload (gelu table resident early) ----
            scr = wrk.tile([1, 2], F32)
            nc.vector.memset(scr[:], 0.0)
            scr2 = wrk.tile([1, 2], F32)
            nc.scalar.activation(scr2[:, 0:1], scr[:, 0:1], AF.Gelu_apprx_tanh)

            # ---- DMA kicks: few, large transfers per ring ----
            # sync ring:   xt, w1s[0:72], dW1A chunks        (~6.9 MB)
            # scalar ring: cons, db1r, w1s[72:96], wd2, dW1B (~6.7 MB)
            # gpsimd:      xtf8 in, fo/po out
            xtf8_sb = cst.tile([128, KD * B], FP8)
            nc.sync.dma_start(xtf8_sb[:], XTF8.ap())
            w1s_sb = cst.tile([128, KD * FSH], BF16)
            nc.sync.dma_start(w1s_sb[:, 0:36 * FSH],
                              W1S.ap()[:, 0:36 * FSH])
            nc.sync.dma_start(w1s_sb[:, 36 * FSH:72 * FSH],
                              W1S.ap()[:, 36 * FSH:72 * FSH])
            dwa = []
            for j in range(NDWC):
                t_ = dwap.tile([128, DWCH[j] * HW], FP8, name="dwat",
                               tag=f"dwat{j}")
                nc.sync.dma_start(
                    t_[:], DW1A.ap()[:, DWOF[j] * HW:DWOF[j + 1] * HW])
                dwa.append(t_)

            cons_sb = cst.tile([128, 35], F32)
            nc.scalar.dma_start(cons_sb[:], CONS.ap())
            db1r_sb = cst.tile([B, F], F32)
            nc.scalar.dma_start(db1r_sb[:], DB1R.ap())
            xt_sb = cst.tile([128, KD * B], BF16)
            nc.scalar.dma_start(xt_sb[:], XT.ap())
            nc.scalar.dma_start(w1s_sb[:, 72 * FSH:KD * FSH],
                                W1S.ap()[:, 72 * FSH:KD * FSH])
            wd2_sb = cst.tile([FSH, (T + 1) * F], BF16)
            nc.scalar.dma_start(wd2_sb[:], WD2.ap())
            dwb = []
            for j in range(NDWC):
                t_ = dwbp.tile([128, DWCH[j] * HW], FP8, name="dwbt",
                               tag=f"dwbt{j}")
                nc.scalar.dma_start(
                    t_[:], DW1B.ap()[:, DWOF[j] * HW:DWOF[j + 1] * HW])
                dwb.append(t_)

            eye = cons_sb[0:32, 0:32]
            b1c = cons_sb[0:FSH, 32:33]
            b1p = cons_sb[0:FSH, 33:34]
            b1m = cons_sb[0:FSH, 34:35]
            w2_sb = wd2_sb[:, 0:F]
            xtf8_3 = xtf8_sb[:].rearrange("p (k b) -> p k b", b=B)

            # ---- z1 = X @ W1[:, fc]  (single accumulation group,
            # k-order follows DMA arrival: scalar-ring k72-95 lands first)
            z1ps = pss.tile([B, FSH], F32, name="sp", tag="sp",
                            padded_shape=[128, 512])
            kseq = list(range(72, KD)) + list(range(0, 72))
            for i, k in enumerate(kseq):
                nc.tensor.matmul(z1ps[:], xt_sb[:, k * B:(k + 1) * B],
                                 w1s_sb[:, k * FSH:(k + 1) * FSH],
                                 start=(i == 0), stop=(i == KD - 1),
                                 skip_group_check=True)

            # z1 -> f-major [f, b] via PE transpose
            z1pad = wrk.tile([32, FSH], F32)
            nc.vector.tensor_copy(z1pad[0:B, :], z1ps[:])
            z1tp = pss.tile([FSH, 32], F32, name="sp", tag="sp",
                            padded_shape=[128, 512])
            nc.tensor.matmul(z1tp[:], z1pad[:], eye, is_transpose=True,
                             skip_group_check=True)
            z1t = wrk.tile([FSH, B], F32)
            nc.vector.tensor_copy(z1t[:], z1tp[:, 0:B])

            # g = gelu(z1+b1); gpd = (G(z1+b1+eps) - G(z1+b1-eps)) * GPD_S
            gT = wrk.tile([FSH, B], BF16)
            nc.scalar.activation(gT[:], z1t[:], AF.Gelu_apprx_tanh, bias=b1c)
            gpp = wrk.tile([FSH, B], F32)
            nc.scalar.activation(gpp[:], z1t[:], AF.Gelu_apprx_tanh, bias=b1p)
            gpm = wrk.tile([FSH, B], F32)
            nc.scalar.activation(gpm[:], z1t[:], AF.Gelu_apprx_tanh, bias=b1m)
            gpd = wrk.tile([FSH, B], F32)
            nc.vector.tensor_sub(gpd[:], gpp[:], gpm[:])
            nc.vector.tensor_scalar_mul(gpd[:], gpd[:], GPD_S)

            # pair stationaries for the g @ dW2 term: [g|0] and [0|g]
            gpe = wrk.tile([FSH, 32], BF16)
            nc.vector.memset(gpe[:], 0.0)
            gpo = wrk.tile([FSH, 32], BF16)
            nc.vector.memset(gpo[:], 0.0)
            nc.vector.tensor_copy(gpe[:, 0:16], gT[:])
            nc.vector.tensor_copy(gpo[:, 16:32], gT[:])

            # ---- feats partial: fo = g @ W2[fc, :] ----
            fps5 = pss.tile([B, 512], F32, name="sp", tag="sp",
                            padded_shape=[128, 512])
            nc.tensor.matmul(fps5[:], gT[:], w2_sb[:, 0:512],
                             start=True, stop=True, skip_group_check=True)
            fps2 = pss.tile([B, 256], F32, name="sp", tag="sp",
                            padded_shape=[128, 512])
            nc.tensor.matmul(fps2[:], gT[:], w2_sb[:, 512:F],
                             start=True, stop=True, skip_group_check=True)
            fo_sb = wrk.tile([B, F], F32)
            nc.vector.tensor_copy(fo_sb[:, 0:512], fps5[:])
            nc.vector.tensor_copy(fo_sb[:, 512:F], fps2[:])
            nc.gpsimd.dma_start(FO.ap(), fo_sb[:])

            # ---- P accumulation: pair group j owns rows 32j:32j+32 ----
            P5 = psu.tile([128, 512], F32, name="p5")
            P2 = psu.tile([128, 256], F32, name="p2",
                          padded_shape=[128, 512])
            vps = wrk.tile([FSH, 128], BF16)
            po_sb = wrk.tile([128, F], BF16)

            def mm_gterm(t):
                j = t // 2
                st = gpe if t % 2 == 0 else gpo
                mv = wd2_sb[:, (1 + t) * F:(1 + t) * F + 512]
                nc.tensor.matmul(P5[32 * j:32 * j + 32, :], st[:], mv,
                                 start=(t % 2 == 0), stop=False,
                                 tile_position=(0, 32 * j),
                                 skip_group_check=True)
                mv = wd2_sb[:, (1 + t) * F + 512:(2 + t) * F]
                nc.tensor.matmul(P2[32 * j:32 * j + 32, :], st[:], mv,
                                 start=(t % 2 == 0), stop=False,
                                 tile_position=(0, 32 * j),
                                 skip_group_check=True)

            for t in range(T):
                mm_gterm(t)

            # ---- U streams (DoubleRow fp8): two k-pair chains per half,
            # each in its own PSUM bank at partition 0 (the ISA requires
            # dst partition 0 for DoubleRow)
            upsA = [psa.tile([B, HW], F32, name=f"ua{i}",
                             padded_shape=[128, 512]) for i in range(2)]
            upsB = [psb.tile([B, HW], F32, name=f"ub{i}",
                             padded_shape=[128, 512]) for i in range(2)]

            def chunk_u(ups, chunks, j):
                ch3 = chunks[j][:].rearrange("p (k f) -> p k f", f=HW)
                for lp in range(DWCH[j] // 2):
                    p = DWOF[j] // 2 + lp
                    nc.tensor.matmul(ups[p % 2][:],
                                     xtf8_3[:, 2 * p:2 * p + 2, :],
                                     ch3[:, 2 * lp:2 * lp + 2, :],
                                     start=(p < 2), stop=(p >= KD // 2 - 2),
                                     perf_mode=PM.DoubleRow,
                                     skip_group_check=True)

            def chunkA(j):
                chunk_u(upsA, dwa, j)

            def chunkB(j):
                chunk_u(upsB, dwb, j)

            def drain_half(ups, tbase):
                # (U0 + 128*db1) + U1, transpose, * gelu'-scale; each add
                # reads one PSUM operand (hardware limit) and one SBUF one
                for tt in range(4):
                    t = tbase + tt
                    tsum = wrk.tile([B, FSH], F32, name="ts", tag="ts",
                                    bufs=2)
                    nc.vector.tensor_add(
                        tsum[:],
                        ups[0][:, tt * FSH:(tt + 1) * FSH],
                        db1r_sb[:, t * FSH:(t + 1) * FSH])
                    tzpad = wrk.tile([32, FSH], F32, name="tz", tag="tz",
                                     bufs=2)
                    nc.vector.tensor_add(
                        tzpad[0:B, :],
                        ups[1][:, tt * FSH:(tt + 1) * FSH],
                        tsum[:])
                    tztp = pss.tile([FSH, 32], F32, name="sp", tag="sp",
                                    padded_shape=[128, 512])
                    nc.tensor.matmul(tztp[:], tzpad[:], eye,
                                     is_transpose=True,
                                     skip_group_check=True)
                    nc.vector.tensor_mul(vps[:, t * B:(t + 1) * B],
                                         tztp[:, 0:B], gpd[:])
                ro = tbase * B
                nc.tensor.matmul(P5[ro:ro + 64, :], vps[:, ro:ro + 64],
                                 w2_sb[:, 0:512],
                                 start=False, stop=True,
                                 tile_position=(0, ro),
                                 skip_group_check=True)
                nc.tensor.matmul(P2[ro:ro + 64, :], vps[:, ro:ro + 64],
                                 w2_sb[:, 512:F],
                                 start=False, stop=True,
                                 tile_position=(0, ro),
                                 skip_group_check=True)
                # export this half's PO rows on its own (now idle) ring
                nc.vector.tensor_copy(po_sb[ro:ro + 64, 0:512],
                                      P5[ro:ro + 64, :])
                nc.vector.tensor_copy(po_sb[ro:ro + 64, 512:F],
                                      P2[ro:ro + 64, :])
                eng = nc.sync if tbase == 0 else nc.scalar
                eng.dma_start(PO.ap()[ro:ro + 64, :], po_sb[ro:ro + 64, :])

            # PE emission order tracks expected data-arrival order
            chunkA(0)
            chunkB(0)
            for j in range(1, 8):
                chunkA(j)
                chunkB(j)
            chunkA(8)
            chunkA(9)
            chunkB(8)
            drain_half(upsA, 0)
            chunkB(9)
            drain_half(upsB, 4)

    nc.compile()
    return nc


def _get_nc():
    if "nc" not in _CACHE:
        _CACHE["nc"] = build()
    return _CACHE["nc"]


def _prep_in_maps(x, W1, b1, W2, b2, mW1, mb1, mW2, mb2, dW1, db1, dW2, db2):
    f32 = np.float32
    bf16 = ml_dtypes.bfloat16
    fp8 = ml_dtypes.float8_e4m3
    X = np.ascontiguousarray(np.asarray(x, f32).reshape(B, D))
    XT = np.ascontiguousarray(X.T)
    XTl = XT.reshape(KD, 128, B).transpose(1, 0, 2).reshape(128, KD * B)
    XTb = np.ascontiguousarray(XTl).astype(bf16)
    XTf8 = np.ascontiguousarray(XTl * FP8_XS).astype(fp8)
    W1 = np.asarray(W1, f32)
    W2 = np.asarray(W2, f32)
    b1 = np.asarray(b1, f32)
    dW1 = np.asarray(dW1, f32)
    db1 = np.asarray(db1, f32)
    dW2 = np.asarray(dW2, f32)

    in_maps = []
    for c in range(NCORES):
        fc = slice(c * FSH, (c + 1) * FSH)
        w1s = np.ascontiguousarray(
            W1[:, fc].reshape(KD, 128, FSH).transpose(1, 0, 2)
            .reshape(128, KD * FSH)).astype(bf16)
        dwa = np.ascontiguousarray(
            (dW1[0:4, :, fc] * FP8_SCALE).transpose(1, 0, 2)
            .reshape(KD, 128, HW).transpose(1, 0, 2)
            .reshape(128, KD * HW)).astype(fp8)
        dwb = np.ascontiguousarray(
            (dW1[4:8, :, fc] * FP8_SCALE).transpose(1, 0, 2)
            .reshape(KD, 128, HW).transpose(1, 0, 2)
            .reshape(128, KD * HW)).astype(fp8)
        wd2 = np.concatenate(
            [W2[fc, :][:, None, :],
             dW2[:, fc, :].transpose(1, 0, 2)], axis=1).reshape(FSH,
                                                               (T + 1) * F)
        cons = np.zeros((128, 35), f32)
        cons[0:32, 0:32] = np.eye(32, dtype=f32)
        cons[0:FSH, 32] = b1[fc]
        cons[0:FSH, 33] = b1[fc] + EPS
        cons[0:FSH, 34] = b1[fc] - EPS
        db1r = np.ascontiguousarray(np.broadcast_to(
            (FP8_SCALE * FP8_XS * db1[:, fc]).reshape(T * FSH), (B, F)))
        in_maps.append({
            "xt": XTb,
            "xtf8": XTf8,
            "w1s": w1s,
            "dw1a": dwa,
            "dw1b": dwb,
            "wd2": np.ascontiguousarray(wd2).astype(bf16),
            "cons": cons,
            "db1r": db1r.astype(f32),
        })
    return in_maps


def run(inputs, trace=False, trace_cores=None, tmpdir=None):
    nc = _get_nc()
    in_maps = _prep_in_maps(**inputs)
    res = bass_utils.run_bass_kernel_spmd(
        nc, in_maps, core_ids=list(range(NCORES)), trace=trace,
        trace_cores=trace_cores, tmpdir=tmpdir)

    f64 = np.float64
    b2 = np.asarray(inputs["b2"], f64)
    mW1 = np.asarray(inputs["mW1"], f64)
    mb1 = np.asarray(inputs["mb1"], f64)
    mW2 = np.asarray(inputs["mW2"], f64)
    mb2 = np.asarray(inputs["mb2"], f64)
    db2 = np.asarray(inputs["db2"], f64)

    feats = np.zeros((B, F), f64)
    P = np.zeros((128, F), f64)
    for c in range(NCORES):
        feats += res.results[c]["fo"].astype(f64)
        P += res.results[c]["po"].astype(f64)
    feats += b2[None, :]
    h = np.maximum(feats @ mW1.T + mb1, 0.0)
    coefs = h @ mW2.T + mb2                     # [B, T]
    out = feats + coefs @ db2
    for t in range(T):
        out += coefs[:, t:t + 1] * P[t * B:(t + 1) * B]
    return out.astype(np.float32), res


def kernel(**inputs):
    out, _ = run(inputs, trace=False)
    return out
